# revision 11
# baseline (speedup 1.0000x reference)
"""GCN 3-layer kernel for Trainium2, 8-core SPMD — v8.

Algebraic restructure: with no inter-layer nonlinearity, the network
collapses to

    out = relu( A^3 (X W1W2W3) + (A^2 1) b1'W2W3 + (A 1) b2'W3 + 1 b3' )

where A = D^-1/2 (Adj^T + I) D^-1/2 is the normalized propagation
operator.  W123 = W1@W2@W3 and the rank-1 bias corrections (v1 = A 1,
v2 = A^2 1) are precomputed on the host, so the device only runs THREE
sparse propagations of a 64-wide table plus one dense 128x64 input
matmul — no per-layer GEMMs, no transposes.

Per propagation round (per dst-shard core, PyG GCN convention):
    y      = dinv * T                  (64-wide bf16 message table)
    agg[d] = sum_{e: dst[e]=d} y[src[e]]
    T_next = dinv * agg + dinv^2 * T   (self-loop folded via s-table)

Distribution: destination-sharded across 8 cores (6272 nodes/core).
The y table rows are stored 256B-strided (64 bf16 payload + 64 garbage
pad) because the SWDGE gather stride field has 256B granularity; the
per-edge gather moves only the 128B payload (raw-emitted
InstDMAGatherAnt — the bass wrapper's %256 payload restriction is a
transpose-mode constraint).  Per layer the table is replicated via TWO
AllGathers over window halves (0-24 / 25-48), issued early so they hide
under the previous rounds's gathers; halving also keeps gather indices
below 2^15 (int16).

Each core gathers message rows for its incoming edges with per-edge
SWDGE descriptors on the four Q7 queue-pairs; messages are scatter-added
with one-hot matmuls on the PE (PSUM accumulation per 128-dst window);
the one-hot S for a whole window is built with a single broadcast
tensor_tensor is_equal.
"""

import numpy as np
import ml_dtypes

N_NODES = 50000
N_CORES = 8
PER_CORE = 6272            # 49 * 128
N_PAD = PER_CORE * N_CORES # 50176
N_WIN = 49
QUARTERS = [(0, 15), (15, 30), (30, 44), (44, 49)]  # table quarters (uneven:
# the last is small so its end-of-round AllGather exposes minimal latency)
NQ = 4
F = 128                    # input feature width / padded row stride
FO = 64                    # propagated table width (= output width)
GROUP_WINDOWS = 5          # windows per compute group
# AG for quarter q fires after this group index (group covers its windows)
AG_AFTER_GROUP = [2, 5, 8, 9]
CHUNK = 2048               # max gather slots per SWDGE call (~desc ring size)

BF16 = ml_dtypes.bfloat16


def _wrap_idx16(idx: np.ndarray) -> np.ndarray:
    """Wrap a flat int16 index stream into the [128, n/16] layout dma_gather
    expects (element i at [i%16, i//16], replicated across the 8 groups of
    16 partitions)."""
    n = len(idx)
    assert n % 128 == 0
    cols = n // 16
    out = np.empty((128, cols), np.int16)
    w = idx.reshape(cols, 16).T  # [16, cols]
    for g in range(8):
        out[g * 16:(g + 1) * 16, :] = w
    return out


def _preprocess(edge_index: np.ndarray):
    """Host-side graph prep: degree norm, dst-sharding, per-(window, half)
    edge streams, block padding shared across cores, and the A^k 1 vectors
    for the rank-1 bias corrections.  Self-loops are NOT materialized as
    edges (handled via the s table)."""
    src = edge_index[0].astype(np.int64)
    dst = edge_index[1].astype(np.int64)
    deg = np.bincount(dst, minlength=N_NODES).astype(np.float64) + 1.0
    dinv64 = 1.0 / np.sqrt(deg)
    dinv = dinv64.astype(np.float32)
    dinv_pad = np.ones(N_PAD, np.float32)
    dinv_pad[:N_NODES] = dinv

    # v1 = A 1, v2 = A v1 (host, fp64)
    def ahat(t):
        agg = np.zeros(N_NODES, np.float64)
        np.add.at(agg, dst, dinv64[src] * t[src])
        return dinv64 * agg + dinv64 * dinv64 * t

    v1 = ahat(np.ones(N_NODES, np.float64))
    v2 = ahat(v1)
    v1_pad = np.zeros(N_PAD, np.float32); v1_pad[:N_NODES] = v1
    v2_pad = np.zeros(N_PAD, np.float32); v2_pad[:N_NODES] = v2

    core_of = dst // PER_CORE
    win_of = (dst % PER_CORE) // 128
    dloc_of = dst % 128

    src_core = src // PER_CORE
    src_off = src % PER_CORE
    src_win = src_off // 128
    q_of = np.searchsorted([q[1] for q in QUARTERS], src_win, side="right")
    rows_q = np.array([(q[1] - q[0]) * 128 for q in QUARTERS])
    w0_q = np.array([q[0] * 128 for q in QUARTERS])
    idx_val = src_core * rows_q[q_of] + (src_off - w0_q[q_of])

    order = np.lexsort((dst, q_of, win_of, core_of))
    core_s, win_s, dloc_s, q_s, iv_s = (
        core_of[order], win_of[order], dloc_of[order], q_of[order],
        idx_val[order])

    # per (core, window, half) counts -> shared block counts
    counts = np.zeros((N_CORES, N_WIN, NQ), np.int64)
    np.add.at(counts, (core_s, win_s, q_s), 1)
    blk = np.maximum(1, -(-counts.max(axis=0) // 128))  # [N_WIN, NQ]

    # per-half stream offsets: stream q holds its blocks window-major
    off = np.zeros((NQ, N_WIN + 1), np.int64)
    for q in range(NQ):
        off[q, 1:] = np.cumsum(blk[:, q] * 128)
    n_q = off[:, -1].astype(int)  # slots per stream

    idx_q = [np.zeros((N_CORES, int(n)), np.int16) for n in n_q]
    dl_q = [np.full((N_CORES, int(n)), 999.0, np.float32) for n in n_q]

    keys = (core_s * N_WIN + win_s) * NQ + q_s
    bounds = np.searchsorted(keys, np.arange(N_CORES * N_WIN * NQ + 1))
    for c in range(N_CORES):
        for w in range(N_WIN):
            for q in range(NQ):
                k = (c * N_WIN + w) * NQ + q
                sl = slice(bounds[k], bounds[k + 1])
                iv = iv_s[sl]; dl = dloc_s[sl]
                o = off[q, w]
                idx_q[q][c, o:o + len(iv)] = iv.astype(np.int16)
                dl_q[q][c, o:o + len(iv)] = dl

    # combined per-window dl (all halves' blocks of window w contiguous),
    # matching the matmul consumption order
    blk_w = blk.sum(axis=1)             # blocks per window
    off_w = np.concatenate([[0], np.cumsum(blk_w)])  # block offsets
    n_blk = int(off_w[-1])
    dl_win = np.full((N_CORES, n_blk * 128), 999.0, np.float32)
    for c in range(N_CORES):
        for w in range(N_WIN):
            o = off_w[w] * 128
            for q in range(NQ):
                nbq = int(blk[w, q]) * 128
                dl_win[c, o:o + nbq] = dl_q[q][c, off[q, w]:off[q, w] + nbq]
                o += nbq

    return (dinv_pad, v1_pad, v2_pad, blk, off, idx_q, dl_win, blk_w, off_w)


def _raw_gather(gp, mybir, out_ap, in_ap, idxs_ap, num_idxs, elem_size,
                elem_step, queue_num):
    """dma_gather without the elem_size_bytes % 256 restriction (a
    transpose-mode constraint); the row stride (elem_step) must still be a
    multiple of 256B due to the 8-bit stride_bytes_256 descriptor field."""
    stride_bytes = elem_step * mybir.dt.size(in_ap.dtype)
    assert stride_bytes % 256 == 0 and stride_bytes // 256 < 256
    _in_ap = gp.lower_ap_dma(in_ap, for_custom_bir_dma=True)
    _idxs_ap = gp.lower_ap(idxs_ap)
    _out_ap = gp.lower_ap(out_ap)
    return gp.add_instruction(
        mybir.InstDMAGatherAnt(
            name=gp.bass.get_next_instruction_name(),
            ins=[*_in_ap, _idxs_ap,
                 gp.lower_val_access(gp.to_reg(num_idxs))],
            outs=[_out_ap],
            transpose=False,
            num_idxs=num_idxs,
            elem_size=elem_size,
            stride_bytes_256=stride_bytes // 256,
            gen_mode=0,
            single_packet=False,
            queue_num=queue_num,
            sbuf_tokens_per_rank=0,
            sbuf_free_dim_per_rank=0,
            sbuf_free_dim_pad_per_rank=0,
            sbuf_byte_offset=0,
        ))


def _build_and_run(inputs_np, dinv_pad, v1_pad, v2_pad, blk, off, idx_q,
                   dl_win, blk_w, off_w, trace=False, sim=False):
    import concourse.bacc as bacc
    import concourse.mybir as mybir
    from concourse.tile import TileContext
    from concourse import bass, bass_utils, library_config

    x = inputs_np["x"]
    W1 = np.asarray(inputs_np["W1"], np.float64)
    W2 = np.asarray(inputs_np["W2"], np.float64)
    W3 = np.asarray(inputs_np["W3"], np.float64)
    b1 = np.asarray(inputs_np["b1"], np.float64)
    b2 = np.asarray(inputs_np["b2"], np.float64)
    b3 = np.asarray(inputs_np["b3"], np.float64)
    W123 = (W1 @ W2 @ W3).astype(np.float32)          # [128, 64]
    c1 = (b1 @ W2 @ W3).astype(np.float32)            # [64]
    c2 = (b2 @ W3).astype(np.float32)                 # [64]
    c3 = b3.astype(np.float32)                        # [64]

    n_q = [int(idx_q[q].shape[1]) for q in range(NQ)]
    n_blk = int(off_w[-1])
    G = GROUP_WINDOWS
    groups = [list(range(g, min(g + G, N_WIN))) for g in range(0, N_WIN, G)]
    rows_q = [(q1 - q0) * 128 for q0, q1 in QUARTERS]

    nc = bacc.Bacc("TRN2", target_bir_lowering=False, debug=False,
                   num_devices=N_CORES, num_swdge_queues=4)
    dt = mybir.dt
    Alu = mybir.AluOpType
    Act = mybir.ActivationFunctionType

    # ---- kernel I/O -----------------------------------------------------
    t_xT = nc.dram_tensor("xT_own", [128, PER_CORE], dt.float32, kind="ExternalInput")
    t_W = nc.dram_tensor("W123", [F, FO], dt.float32, kind="ExternalInput")
    t_cb = nc.dram_tensor("cb", [128, 3 * FO], dt.float32, kind="ExternalInput")
    t_dinv = nc.dram_tensor("dinv_own", [128, N_WIN], dt.float32, kind="ExternalInput")
    t_dinv2 = nc.dram_tensor("dinv2_own", [128, N_WIN], dt.float32, kind="ExternalInput")
    t_v1 = nc.dram_tensor("v1_own", [128, N_WIN], dt.float32, kind="ExternalInput")
    t_v2 = nc.dram_tensor("v2_own", [128, N_WIN], dt.float32, kind="ExternalInput")
    t_iota = nc.dram_tensor("iota", [128, 128], dt.bfloat16, kind="ExternalInput")
    t_iq = [nc.dram_tensor(f"idx_q{q}", [128, n_q[q] // 16], dt.int16,
                           kind="ExternalInput") for q in range(NQ)]
    t_dlw = nc.dram_tensor("dl_win", [128, n_blk], dt.bfloat16, kind="ExternalInput")
    t_out = nc.dram_tensor("h_out", [PER_CORE, FO], dt.float32, kind="ExternalOutput")

    with TileContext(nc) as tc:
        nc.gpsimd.load_library(library_config.mlp)
        with tc.tile_pool(name="const", bufs=1) as cpool, \
             tc.tile_pool(name="state", bufs=1) as spool, \
             tc.tile_pool(name="gath", bufs=3) as gpool, \
             tc.tile_pool(name="sbld", bufs=6) as sbld, \
             tc.tile_pool(name="work", bufs=3) as wpool, \
             tc.tile_pool(name="stg", bufs=2) as stg, \
             tc.tile_pool(name="psA", bufs=4, space="PSUM") as psA, \
             tc.tile_pool(name="dram", bufs=1, space="DRAM") as dpool:

            # ---- constants ----
            c_W = cpool.tile([F, FO], dt.float32, tag="W", name="W123")
            c_cb = cpool.tile([128, 3 * FO], dt.float32, tag="cb", name="cb")
            c_dinv = cpool.tile([128, N_WIN], dt.float32, tag="dinv", name="dinv")
            c_dinv2 = cpool.tile([128, N_WIN], dt.float32, tag="dinv2", name="dinv2")
            c_v1 = cpool.tile([128, N_WIN], dt.float32, tag="v1", name="v1")
            c_v2 = cpool.tile([128, N_WIN], dt.float32, tag="v2", name="v2")
            c_iota = cpool.tile([128, 128], dt.bfloat16, tag="iota", name="iota")
            c_iq = [cpool.tile([128, n_q[q] // 16], dt.int16, tag=f"iq{q}",
                               name=f"iq{q}") for q in range(NQ)]
            c_dlw = cpool.tile([128, n_blk], dt.bfloat16, tag="dlw", name="dlw")
            nc.sync.dma_start(c_W[:], t_W[:])
            nc.sync.dma_start(c_cb[:], t_cb[:])
            nc.sync.dma_start(c_dinv[:], t_dinv[:])
            nc.sync.dma_start(c_dinv2[:], t_dinv2[:])
            nc.sync.dma_start(c_v1[:], t_v1[:])
            nc.sync.dma_start(c_v2[:], t_v2[:])
            nc.sync.dma_start(c_iota[:], t_iota[:])
            for q in range(NQ):
                nc.sync.dma_start(c_iq[q][:], t_iq[q][:])
            nc.sync.dma_start(c_dlw[:], t_dlw[:])

            # ---- persistent state: s = dinv^2 * T (+c3 for round 3) ------
            s_tab = [spool.tile([128, N_WIN, FO], dt.float32, tag="s_a", name="s_a"),
                     spool.tile([128, N_WIN, FO], dt.float32, tag="s_b", name="s_b")]

            # y tables: rows 256B-strided, payload = first 64 bf16
            y_full = [[dpool.tile([N_CORES * rows_q[q], F], dt.bfloat16,
                                  addr_space="Shared", name=f"y_full{i}_{q}")
                       for q in range(NQ)] for i in range(3)]
            ag_in = [[dpool.tile([rows_q[q], F], dt.bfloat16, name=f"ag_in{i}_{q}")
                      for q in range(NQ)] for i in range(3)]

            def flush_y(g, r_next, yst):
                """DMA the staged bf16 y rows of group g to the AG inputs.
                A group may straddle a half boundary."""
                w0, w1 = g[0], g[-1] + 1
                s = w0
                while s < w1:
                    q = next(i for i, (a, b) in enumerate(QUARTERS)
                             if a <= s < b)
                    e = min(w1, QUARTERS[q][1])
                    dst = ag_in[r_next][q][
                        (s - QUARTERS[q][0]) * 128:(e - QUARTERS[q][0]) * 128, :]
                    nc.sync.dma_start(dst.rearrange("(t p) f -> p t f", p=128),
                                      yst[:, s - w0:e - w0, :])
                    s = e

            def ag_half(r, q):
                nc.gpsimd.collective_compute(
                    "AllGather", Alu.bypass,
                    replica_groups=[list(range(N_CORES))],
                    ins=[ag_in[r][q].opt()], outs=[y_full[r][q].opt()])

            # ---- phase 0: T1 = A-ready tables from Z = x @ W123 ----------
            with tc.tile_pool(name="xp", bufs=1) as xpool:
                xT = xpool.tile([128, PER_CORE], dt.float32, tag="xT", name="xT")
                nc.sync.dma_start(xT[:], t_xT[:])
                nags = 0
                for gi, g in enumerate(groups):
                    yst = stg.tile([128, G, F], dt.bfloat16, tag="yst", name="yst")
                    for wi, w in enumerate(g):
                        ps = psA.tile([128, FO], dt.float32, tag="psA", space="PSUM")
                        nc.tensor.matmul(ps[:], lhsT=xT[:, w * 128:(w + 1) * 128],
                                         rhs=c_W[:], start=True, stop=True)
                        # y1 = dinv * Z (bf16 payload), s1 = dinv^2 * Z
                        nc.scalar.mul(yst[:, wi, :FO], ps[:], c_dinv[:, w:w + 1])
                        nc.scalar.mul(s_tab[1][:, w, :], ps[:], c_dinv2[:, w:w + 1])
                    flush_y(g, 0, yst)
                    while nags < NQ and AG_AFTER_GROUP[nags] == gi:
                        ag_half(0, nags)
                        nags += 1

            # ---- propagation rounds --------------------------------------
            qctr = 0
            for r in range(3):
                nags = 0
                next_slot = [0] * NQ
                calls = [[] for _ in range(NQ)]  # (s0, s1, tile)
                for gi, g in enumerate(groups):
                    # issue gather calls covering this group, split to <=CHUNK
                    for q in range(NQ):
                        end_slot = int(off[q, g[-1] + 1])
                        while next_slot[q] < end_slot:
                            s0 = next_slot[q]
                            n_rem = end_slot - s0
                            n_pieces = -(-n_rem // CHUNK)
                            sz = -(-(n_rem // 128) // n_pieces) * 128
                            s1 = min(s0 + sz, end_slot)
                            t = gpool.tile([128, CHUNK // 128, FO], dt.bfloat16,
                                           tag=f"m{q}_{len(calls[q]) % 3}",
                                           name=f"m{r}_{q}_{len(calls[q])}")
                            _raw_gather(
                                nc.gpsimd, mybir,
                                t[:, :(s1 - s0) // 128, :],
                                y_full[r][q][:, :FO],
                                c_iq[q][:, s0 // 16:s1 // 16],
                                s1 - s0, FO, F, qctr % 4)
                            calls[q].append((s0, s1, t))
                            next_slot[q] = s1
                            qctr += 1
                    yst = stg.tile([128, G, F], dt.bfloat16, tag="yst", name="yst")
                    ost = stg.tile([128, G, FO], dt.float32, tag="ost", name="ost")
                    for wi, w in enumerate(g):
                        nblk = int(blk_w[w])
                        B0 = int(off_w[w])
                        # one-hot S for the whole window in one op
                        S = sbld.tile([128, nblk, 128], dt.bfloat16, tag="S", name="S")
                        dl_b = (c_dlw[:, B0:B0 + nblk].unsqueeze(2)
                                .broadcast_to([128, nblk, 128]))
                        nc.vector.tensor_tensor(
                            out=S[:, :, :], in0=dl_b,
                            in1=c_iota[:].unsqueeze(1).broadcast_to([128, nblk, 128]),
                            op=Alu.is_equal)
                        # scatter-add via PSUM-accumulated one-hot matmuls
                        agg = psA.tile([128, FO], dt.float32, tag="psA", space="PSUM")
                        k = 0
                        for q in range(NQ):
                            for b in range(int(blk[w, q])):
                                gslot = int(off[q, w]) + b * 128
                                s0, s1, t = next(
                                    c for c in reversed(calls[q])
                                    if c[0] <= gslot < c[1])
                                nc.tensor.matmul(
                                    agg[:], lhsT=S[:, k, :],
                                    rhs=t[:, (gslot - s0) // 128, :],
                                    start=(k == 0), stop=(k == nblk - 1))
                                k += 1
                        # ---- epilogue: T = dinv*agg + s ----
                        h = wpool.tile([128, FO], dt.float32, tag="h", name="h")
                        nc.vector.scalar_tensor_tensor(
                            out=h[:], in0=agg[:], scalar=c_dinv[:, w:w + 1],
                            in1=s_tab[(r + 1) % 2][:, w, :],
                            op0=Alu.mult, op1=Alu.add)
                        if r < 2:
                            nc.scalar.mul(yst[:, wi, :FO], h[:], c_dinv[:, w:w + 1])
                            if r == 0:
                                nc.scalar.mul(s_tab[0][:, w, :], h[:],
                                              c_dinv2[:, w:w + 1])
                            else:
                                # s3 = dinv^2*T3 + c3 (bias const folded in)
                                nc.vector.scalar_tensor_tensor(
                                    out=s_tab[1][:, w, :], in0=h[:],
                                    scalar=c_dinv2[:, w:w + 1],
                                    in1=c_cb[:, 2 * FO:3 * FO],
                                    op0=Alu.mult, op1=Alu.add)
                        else:
                            # out = relu(T4 + v1*c2 + v2*c1)
                            f1 = wpool.tile([128, FO], dt.float32, tag="f1", name="f1")
                            nc.vector.scalar_tensor_tensor(
                                out=f1[:], in0=c_cb[:, FO:2 * FO],
                                scalar=c_v1[:, w:w + 1], in1=h[:],
                                op0=Alu.mult, op1=Alu.add)
                            f2 = wpool.tile([128, FO], dt.float32, tag="f2", name="f2")
                            nc.vector.scalar_tensor_tensor(
                                out=f2[:], in0=c_cb[:, 0:FO],
                                scalar=c_v2[:, w:w + 1], in1=f1[:],
                                op0=Alu.mult, op1=Alu.add)
                            nc.scalar.activation(ost[:, wi, :], f2[:], Act.Relu)
                    if r < 2:
                        flush_y(g, r + 1, yst)
                        while nags < NQ and AG_AFTER_GROUP[nags] == gi:
                            ag_half(r + 1, nags)
                            nags += 1
                    else:
                        w0, w1 = g[0], g[-1] + 1
                        nc.sync.dma_start(
                            t_out[w0 * 128:w1 * 128, :]
                            .rearrange("(t p) f -> p t f", p=128),
                            ost[:, :w1 - w0, :])

    nc.compile()

    # ---- per-core inputs ----
    xT_all = np.zeros((128, N_PAD), np.float32)
    xT_all[:, :N_NODES] = np.asarray(x, np.float32).T
    iota_m = np.broadcast_to(np.arange(128, dtype=np.float32), (128, 128)).astype(BF16)
    cb = np.zeros((128, 3 * FO), np.float32)
    cb[:, 0:FO] = c1; cb[:, FO:2 * FO] = c2; cb[:, 2 * FO:3 * FO] = c3
    in_maps = []
    for c in range(N_CORES):
        rows = slice(c * PER_CORE, (c + 1) * PER_CORE)
        din = dinv_pad[rows].reshape(N_WIN, 128).T.copy()  # [128, N_WIN]
        in_map = {
            "xT_own": np.ascontiguousarray(xT_all[:, rows]),
            "dinv_own": din,
            "dinv2_own": din * din,
            "v1_own": v1_pad[rows].reshape(N_WIN, 128).T.copy(),
            "v2_own": v2_pad[rows].reshape(N_WIN, 128).T.copy(),
            "iota": iota_m.copy(),
            "dl_win": dl_win[c].reshape(-1, 128).T.astype(BF16).copy(),
            "W123": W123.copy(),
            "cb": cb.copy(),
        }
        for q in range(NQ):
            in_map[f"idx_q{q}"] = _wrap_idx16(idx_q[q][c])
        in_maps.append(in_map)

    if sim:
        from concourse.bass_interp import MultiCoreSim
        mcs = MultiCoreSim(nc, num_cores=N_CORES, trace=False,
                           require_finite=False, require_nnan=False)
        for ci, core in enumerate(mcs.cores.values()):
            for k, v in in_maps[ci].items():
                core.tensor(k)[:] = v
        mcs.simulate(check_with_hw=False)
        outs = [np.asarray(core.tensor("h_out"))
                for core in mcs.cores.values()]
        res = None
    else:
        res = bass_utils.run_bass_kernel_spmd(
            nc, in_maps, core_ids=list(range(N_CORES)), trace=trace)
        outs = [r["h_out"] for r in res.results]
    full = np.concatenate(outs, axis=0)[:N_NODES]
    return full, res


def kernel(**inputs) -> np.ndarray:
    edge_index = np.asarray(inputs["edge_index"])
    prep = _preprocess(edge_index)
    out, _ = _build_and_run(inputs, *prep)
    return out


# revision 15
# speedup vs baseline: 1.0116x; 1.0116x over previous
"""GCN 3-layer kernel for Trainium2, 8-core SPMD — v8.

Algebraic restructure: with no inter-layer nonlinearity, the network
collapses to

    out = relu( A^3 (X W1W2W3) + (A^2 1) b1'W2W3 + (A 1) b2'W3 + 1 b3' )

where A = D^-1/2 (Adj^T + I) D^-1/2 is the normalized propagation
operator.  W123 = W1@W2@W3 and the rank-1 bias corrections (v1 = A 1,
v2 = A^2 1) are precomputed on the host, so the device only runs THREE
sparse propagations of a 64-wide table plus one dense 128x64 input
matmul — no per-layer GEMMs, no transposes.

Per propagation round (per dst-shard core, PyG GCN convention):
    y      = dinv * T                  (64-wide bf16 message table)
    agg[d] = sum_{e: dst[e]=d} y[src[e]]
    T_next = dinv * agg + dinv^2 * T   (self-loop folded via s-table)

Distribution: destination-sharded across 8 cores (6272 nodes/core).
The y table rows are stored 256B-strided (64 bf16 payload + 64 garbage
pad) because the SWDGE gather stride field has 256B granularity; the
per-edge gather moves only the 128B payload (raw-emitted
InstDMAGatherAnt — the bass wrapper's %256 payload restriction is a
transpose-mode constraint).  Per layer the table is replicated via TWO
AllGathers over window halves (0-24 / 25-48), issued early so they hide
under the previous rounds's gathers; halving also keeps gather indices
below 2^15 (int16).

Each core gathers message rows for its incoming edges with per-edge
SWDGE descriptors on the four Q7 queue-pairs; messages are scatter-added
with one-hot matmuls on the PE (PSUM accumulation per 128-dst window);
the one-hot S for a whole window is built with a single broadcast
tensor_tensor is_equal.
"""

import numpy as np
import ml_dtypes

N_NODES = 50000
N_CORES = 8
PER_CORE = 6272            # 49 * 128
N_PAD = PER_CORE * N_CORES # 50176
N_WIN = 49
QUARTERS = [(0, 25), (25, 44), (44, 49)]  # table slices (uneven: the last is
# small so its end-of-round AllGather exposes minimal latency)
NQ = 3
F = 128                    # input feature width / padded row stride
FO = 64                    # propagated table width (= output width)
GROUP_WINDOWS = 5          # windows per compute group
# AG for quarter q fires after this group index (group covers its windows)
AG_AFTER_GROUP = [4, 8, 9]

BF16 = ml_dtypes.bfloat16


def _wrap_idx16(idx: np.ndarray) -> np.ndarray:
    """Wrap a flat int16 index stream into the [128, n/16] layout dma_gather
    expects (element i at [i%16, i//16], replicated across the 8 groups of
    16 partitions)."""
    n = len(idx)
    assert n % 128 == 0
    cols = n // 16
    out = np.empty((128, cols), np.int16)
    w = idx.reshape(cols, 16).T  # [16, cols]
    for g in range(8):
        out[g * 16:(g + 1) * 16, :] = w
    return out


def _preprocess(edge_index: np.ndarray):
    """Host-side graph prep: degree norm, dst-sharding, per-(window, half)
    edge streams, block padding shared across cores, and the A^k 1 vectors
    for the rank-1 bias corrections.  Self-loops are NOT materialized as
    edges (handled via the s table)."""
    src = edge_index[0].astype(np.int64)
    dst = edge_index[1].astype(np.int64)
    deg = np.bincount(dst, minlength=N_NODES).astype(np.float64) + 1.0
    dinv64 = 1.0 / np.sqrt(deg)
    dinv = dinv64.astype(np.float32)
    dinv_pad = np.ones(N_PAD, np.float32)
    dinv_pad[:N_NODES] = dinv

    # v1 = A 1, v2 = A v1 (host, fp64)
    def ahat(t):
        agg = np.zeros(N_NODES, np.float64)
        np.add.at(agg, dst, dinv64[src] * t[src])
        return dinv64 * agg + dinv64 * dinv64 * t

    v1 = ahat(np.ones(N_NODES, np.float64))
    v2 = ahat(v1)
    v1_pad = np.zeros(N_PAD, np.float32); v1_pad[:N_NODES] = v1
    v2_pad = np.zeros(N_PAD, np.float32); v2_pad[:N_NODES] = v2

    core_of = dst // PER_CORE
    win_of = (dst % PER_CORE) // 128
    dloc_of = dst % 128

    src_core = src // PER_CORE
    src_off = src % PER_CORE
    src_win = src_off // 128
    q_of = np.searchsorted([q[1] for q in QUARTERS], src_win, side="right")
    rows_q = np.array([(q[1] - q[0]) * 128 for q in QUARTERS])
    w0_q = np.array([q[0] * 128 for q in QUARTERS])
    idx_val = src_core * rows_q[q_of] + (src_off - w0_q[q_of])

    order = np.lexsort((dst, q_of, win_of, core_of))
    core_s, win_s, dloc_s, q_s, iv_s = (
        core_of[order], win_of[order], dloc_of[order], q_of[order],
        idx_val[order])

    # per (core, window, half) counts -> shared block counts
    counts = np.zeros((N_CORES, N_WIN, NQ), np.int64)
    np.add.at(counts, (core_s, win_s, q_s), 1)
    blk = np.maximum(1, -(-counts.max(axis=0) // 128))  # [N_WIN, NQ]

    # per-half stream offsets: stream q holds its blocks window-major
    off = np.zeros((NQ, N_WIN + 1), np.int64)
    for q in range(NQ):
        off[q, 1:] = np.cumsum(blk[:, q] * 128)
    n_q = off[:, -1].astype(int)  # slots per stream

    idx_q = [np.zeros((N_CORES, int(n)), np.int16) for n in n_q]
    dl_q = [np.full((N_CORES, int(n)), 999.0, np.float32) for n in n_q]

    keys = (core_s * N_WIN + win_s) * NQ + q_s
    bounds = np.searchsorted(keys, np.arange(N_CORES * N_WIN * NQ + 1))
    for c in range(N_CORES):
        for w in range(N_WIN):
            for q in range(NQ):
                k = (c * N_WIN + w) * NQ + q
                sl = slice(bounds[k], bounds[k + 1])
                iv = iv_s[sl]; dl = dloc_s[sl]
                o = off[q, w]
                idx_q[q][c, o:o + len(iv)] = iv.astype(np.int16)
                dl_q[q][c, o:o + len(iv)] = dl

    # combined per-window dl (all halves' blocks of window w contiguous),
    # matching the matmul consumption order
    blk_w = blk.sum(axis=1)             # blocks per window
    off_w = np.concatenate([[0], np.cumsum(blk_w)])  # block offsets
    n_blk = int(off_w[-1])
    dl_win = np.full((N_CORES, n_blk * 128), 999.0, np.float32)
    for c in range(N_CORES):
        for w in range(N_WIN):
            o = off_w[w] * 128
            for q in range(NQ):
                nbq = int(blk[w, q]) * 128
                dl_win[c, o:o + nbq] = dl_q[q][c, off[q, w]:off[q, w] + nbq]
                o += nbq

    return (dinv_pad, v1_pad, v2_pad, blk, off, idx_q, dl_win, blk_w, off_w)


def _raw_gather(gp, mybir, out_ap, in_ap, idxs_ap, num_idxs, elem_size,
                elem_step, queue_num):
    """dma_gather without the elem_size_bytes % 256 restriction (a
    transpose-mode constraint); the row stride (elem_step) must still be a
    multiple of 256B due to the 8-bit stride_bytes_256 descriptor field."""
    stride_bytes = elem_step * mybir.dt.size(in_ap.dtype)
    assert stride_bytes % 256 == 0 and stride_bytes // 256 < 256
    _in_ap = gp.lower_ap_dma(in_ap, for_custom_bir_dma=True)
    _idxs_ap = gp.lower_ap(idxs_ap)
    _out_ap = gp.lower_ap(out_ap)
    return gp.add_instruction(
        mybir.InstDMAGatherAnt(
            name=gp.bass.get_next_instruction_name(),
            ins=[*_in_ap, _idxs_ap,
                 gp.lower_val_access(gp.to_reg(num_idxs))],
            outs=[_out_ap],
            transpose=False,
            num_idxs=num_idxs,
            elem_size=elem_size,
            stride_bytes_256=stride_bytes // 256,
            gen_mode=0,
            single_packet=False,
            queue_num=queue_num,
            sbuf_tokens_per_rank=0,
            sbuf_free_dim_per_rank=0,
            sbuf_free_dim_pad_per_rank=0,
            sbuf_byte_offset=0,
        ))


def _build_and_run(inputs_np, dinv_pad, v1_pad, v2_pad, blk, off, idx_q,
                   dl_win, blk_w, off_w, trace=False, sim=False):
    import concourse.bacc as bacc
    import concourse.mybir as mybir
    from concourse.tile import TileContext
    from concourse import bass, bass_utils, library_config

    x = inputs_np["x"]
    W1 = np.asarray(inputs_np["W1"], np.float64)
    W2 = np.asarray(inputs_np["W2"], np.float64)
    W3 = np.asarray(inputs_np["W3"], np.float64)
    b1 = np.asarray(inputs_np["b1"], np.float64)
    b2 = np.asarray(inputs_np["b2"], np.float64)
    b3 = np.asarray(inputs_np["b3"], np.float64)
    W123 = (W1 @ W2 @ W3).astype(np.float32)          # [128, 64]
    c1 = (b1 @ W2 @ W3).astype(np.float32)            # [64]
    c2 = (b2 @ W3).astype(np.float32)                 # [64]
    c3 = b3.astype(np.float32)                        # [64]

    n_q = [int(idx_q[q].shape[1]) for q in range(NQ)]
    n_blk = int(off_w[-1])
    G = GROUP_WINDOWS
    groups = [list(range(g, min(g + G, N_WIN))) for g in range(0, N_WIN, G)]
    rows_q = [(q1 - q0) * 128 for q0, q1 in QUARTERS]
    # max slots of stream q in any one group (gather-call tile capacity)
    cap_q = [max(int(off[q, g[-1] + 1] - off[q, g[0]]) for g in groups) // 128
             for q in range(NQ)]

    nc = bacc.Bacc("TRN2", target_bir_lowering=False, debug=False,
                   num_devices=N_CORES, num_swdge_queues=4)
    dt = mybir.dt
    Alu = mybir.AluOpType
    Act = mybir.ActivationFunctionType

    # ---- kernel I/O -----------------------------------------------------
    t_xT = nc.dram_tensor("xT_own", [128, PER_CORE], dt.float32, kind="ExternalInput")
    t_W = nc.dram_tensor("W123", [F, FO], dt.float32, kind="ExternalInput")
    t_cb = nc.dram_tensor("cb", [128, 3 * FO], dt.float32, kind="ExternalInput")
    t_dinv = nc.dram_tensor("dinv_own", [128, N_WIN], dt.float32, kind="ExternalInput")
    t_dinv2 = nc.dram_tensor("dinv2_own", [128, N_WIN], dt.float32, kind="ExternalInput")
    t_v1 = nc.dram_tensor("v1_own", [128, N_WIN], dt.float32, kind="ExternalInput")
    t_v2 = nc.dram_tensor("v2_own", [128, N_WIN], dt.float32, kind="ExternalInput")
    t_iota = nc.dram_tensor("iota", [128, 128], dt.bfloat16, kind="ExternalInput")
    t_iq = [nc.dram_tensor(f"idx_q{q}", [128, n_q[q] // 16], dt.int16,
                           kind="ExternalInput") for q in range(NQ)]
    t_dlw = nc.dram_tensor("dl_win", [128, n_blk], dt.bfloat16, kind="ExternalInput")
    t_out = nc.dram_tensor("h_out", [PER_CORE, FO], dt.float32, kind="ExternalOutput")

    with TileContext(nc) as tc:
        nc.gpsimd.load_library(library_config.mlp)
        with tc.tile_pool(name="const", bufs=1) as cpool, \
             tc.tile_pool(name="state", bufs=1) as spool, \
             tc.tile_pool(name="gath", bufs=3) as gpool, \
             tc.tile_pool(name="sbld", bufs=6) as sbld, \
             tc.tile_pool(name="work", bufs=3) as wpool, \
             tc.tile_pool(name="stg", bufs=2) as stg, \
             tc.tile_pool(name="psA", bufs=4, space="PSUM") as psA, \
             tc.tile_pool(name="dram", bufs=1, space="DRAM") as dpool:

            # ---- constants ----
            c_W = cpool.tile([F, FO], dt.float32, tag="W", name="W123")
            c_cb = cpool.tile([128, 3 * FO], dt.float32, tag="cb", name="cb")
            c_dinv = cpool.tile([128, N_WIN], dt.float32, tag="dinv", name="dinv")
            c_dinv2 = cpool.tile([128, N_WIN], dt.float32, tag="dinv2", name="dinv2")
            c_v1 = cpool.tile([128, N_WIN], dt.float32, tag="v1", name="v1")
            c_v2 = cpool.tile([128, N_WIN], dt.float32, tag="v2", name="v2")
            c_iota = cpool.tile([128, 128], dt.bfloat16, tag="iota", name="iota")
            c_iq = [cpool.tile([128, n_q[q] // 16], dt.int16, tag=f"iq{q}",
                               name=f"iq{q}") for q in range(NQ)]
            c_dlw = cpool.tile([128, n_blk], dt.bfloat16, tag="dlw", name="dlw")
            nc.sync.dma_start(c_W[:], t_W[:])
            nc.sync.dma_start(c_cb[:], t_cb[:])
            nc.sync.dma_start(c_dinv[:], t_dinv[:])
            nc.sync.dma_start(c_dinv2[:], t_dinv2[:])
            nc.sync.dma_start(c_v1[:], t_v1[:])
            nc.sync.dma_start(c_v2[:], t_v2[:])
            nc.sync.dma_start(c_iota[:], t_iota[:])
            for q in range(NQ):
                nc.sync.dma_start(c_iq[q][:], t_iq[q][:])
            nc.sync.dma_start(c_dlw[:], t_dlw[:])

            # ---- persistent state: s = dinv^2 * T (+c3 for round 3) ------
            s_tab = [spool.tile([128, N_WIN, FO], dt.float32, tag="s_a", name="s_a"),
                     spool.tile([128, N_WIN, FO], dt.float32, tag="s_b", name="s_b")]

            # y tables: rows 256B-strided, payload = first 64 bf16
            y_full = [[dpool.tile([N_CORES * rows_q[q], F], dt.bfloat16,
                                  addr_space="Shared", name=f"y_full{i}_{q}")
                       for q in range(NQ)] for i in range(3)]
            ag_in = [[dpool.tile([rows_q[q], F], dt.bfloat16, name=f"ag_in{i}_{q}")
                      for q in range(NQ)] for i in range(3)]

            def flush_y(g, r_next, yst):
                """DMA the staged bf16 y rows of group g to the AG inputs.
                A group may straddle a half boundary."""
                w0, w1 = g[0], g[-1] + 1
                s = w0
                while s < w1:
                    q = next(i for i, (a, b) in enumerate(QUARTERS)
                             if a <= s < b)
                    e = min(w1, QUARTERS[q][1])
                    dst = ag_in[r_next][q][
                        (s - QUARTERS[q][0]) * 128:(e - QUARTERS[q][0]) * 128, :]
                    nc.sync.dma_start(dst.rearrange("(t p) f -> p t f", p=128),
                                      yst[:, s - w0:e - w0, :])
                    s = e

            def ag_half(r, q):
                nc.gpsimd.collective_compute(
                    "AllGather", Alu.bypass,
                    replica_groups=[list(range(N_CORES))],
                    ins=[ag_in[r][q].opt()], outs=[y_full[r][q].opt()])

            # ---- phase 0: T1 = A-ready tables from Z = x @ W123 ----------
            with tc.tile_pool(name="xp", bufs=1) as xpool:
                xT = xpool.tile([128, PER_CORE], dt.float32, tag="xT", name="xT")
                nc.sync.dma_start(xT[:], t_xT[:])
                nags = 0
                for gi, g in enumerate(groups):
                    yst = stg.tile([128, G, F], dt.bfloat16, tag="yst", name="yst")
                    for wi, w in enumerate(g):
                        ps = psA.tile([128, FO], dt.float32, tag="psA", space="PSUM")
                        nc.tensor.matmul(ps[:], lhsT=xT[:, w * 128:(w + 1) * 128],
                                         rhs=c_W[:], start=True, stop=True)
                        # y1 = dinv * Z (bf16 payload), s1 = dinv^2 * Z
                        nc.scalar.mul(yst[:, wi, :FO], ps[:], c_dinv[:, w:w + 1])
                        nc.scalar.mul(s_tab[1][:, w, :], ps[:], c_dinv2[:, w:w + 1])
                    flush_y(g, 0, yst)
                    while nags < NQ and AG_AFTER_GROUP[nags] == gi:
                        ag_half(0, nags)
                        nags += 1

            # ---- propagation rounds --------------------------------------
            qctr = 0
            for r in range(3):
                nags = 0
                next_slot = [0] * NQ
                calls = [[] for _ in range(NQ)]  # (s0, s1, tile)
                for gi, g in enumerate(groups):
                    # one gather call per (group, table slice)
                    for q in range(NQ):
                        end_slot = int(off[q, g[-1] + 1])
                        if next_slot[q] < end_slot:
                            s0 = next_slot[q]
                            s1 = end_slot
                            t = gpool.tile([128, cap_q[q], FO], dt.bfloat16,
                                           tag=f"m{q}_{len(calls[q]) % 3}",
                                           name=f"m{r}_{q}_{len(calls[q])}")
                            _raw_gather(
                                nc.gpsimd, mybir,
                                t[:, :(s1 - s0) // 128, :],
                                y_full[r][q][:, :FO],
                                c_iq[q][:, s0 // 16:s1 // 16],
                                s1 - s0, FO, F, qctr % 4)
                            calls[q].append((s0, s1, t))
                            next_slot[q] = s1
                            qctr += 1
                    yst = stg.tile([128, G, F], dt.bfloat16, tag="yst", name="yst")
                    ost = stg.tile([128, G, FO], dt.float32, tag="ost", name="ost")
                    for wi, w in enumerate(g):
                        nblk = int(blk_w[w])
                        B0 = int(off_w[w])
                        # one-hot S for the whole window in one op
                        S = sbld.tile([128, nblk, 128], dt.bfloat16, tag="S", name="S")
                        dl_b = (c_dlw[:, B0:B0 + nblk].unsqueeze(2)
                                .broadcast_to([128, nblk, 128]))
                        nc.vector.tensor_tensor(
                            out=S[:, :, :], in0=dl_b,
                            in1=c_iota[:].unsqueeze(1).broadcast_to([128, nblk, 128]),
                            op=Alu.is_equal)
                        # scatter-add via PSUM-accumulated one-hot matmuls
                        agg = psA.tile([128, FO], dt.float32, tag="psA", space="PSUM")
                        k = 0
                        for q in range(NQ):
                            for b in range(int(blk[w, q])):
                                gslot = int(off[q, w]) + b * 128
                                s0, s1, t = next(
                                    c for c in reversed(calls[q])
                                    if c[0] <= gslot < c[1])
                                nc.tensor.matmul(
                                    agg[:], lhsT=S[:, k, :],
                                    rhs=t[:, (gslot - s0) // 128, :],
                                    start=(k == 0), stop=(k == nblk - 1))
                                k += 1
                        # ---- epilogue: T = dinv*agg + s ----
                        h = wpool.tile([128, FO], dt.float32, tag="h", name="h")
                        nc.vector.scalar_tensor_tensor(
                            out=h[:], in0=agg[:], scalar=c_dinv[:, w:w + 1],
                            in1=s_tab[(r + 1) % 2][:, w, :],
                            op0=Alu.mult, op1=Alu.add)
                        if r < 2:
                            nc.scalar.mul(yst[:, wi, :FO], h[:], c_dinv[:, w:w + 1])
                            if r == 0:
                                nc.scalar.mul(s_tab[0][:, w, :], h[:],
                                              c_dinv2[:, w:w + 1])
                            else:
                                # s3 = dinv^2*T3 + c3 (bias const folded in)
                                nc.vector.scalar_tensor_tensor(
                                    out=s_tab[1][:, w, :], in0=h[:],
                                    scalar=c_dinv2[:, w:w + 1],
                                    in1=c_cb[:, 2 * FO:3 * FO],
                                    op0=Alu.mult, op1=Alu.add)
                        else:
                            # out = relu(T4 + v1*c2 + v2*c1)
                            f1 = wpool.tile([128, FO], dt.float32, tag="f1", name="f1")
                            nc.vector.scalar_tensor_tensor(
                                out=f1[:], in0=c_cb[:, FO:2 * FO],
                                scalar=c_v1[:, w:w + 1], in1=h[:],
                                op0=Alu.mult, op1=Alu.add)
                            f2 = wpool.tile([128, FO], dt.float32, tag="f2", name="f2")
                            nc.vector.scalar_tensor_tensor(
                                out=f2[:], in0=c_cb[:, 0:FO],
                                scalar=c_v2[:, w:w + 1], in1=f1[:],
                                op0=Alu.mult, op1=Alu.add)
                            nc.scalar.activation(ost[:, wi, :], f2[:], Act.Relu)
                    if r < 2:
                        flush_y(g, r + 1, yst)
                        while nags < NQ and AG_AFTER_GROUP[nags] == gi:
                            ag_half(r + 1, nags)
                            nags += 1
                    else:
                        w0, w1 = g[0], g[-1] + 1
                        nc.sync.dma_start(
                            t_out[w0 * 128:w1 * 128, :]
                            .rearrange("(t p) f -> p t f", p=128),
                            ost[:, :w1 - w0, :])

    nc.compile()

    # ---- per-core inputs ----
    xT_all = np.zeros((128, N_PAD), np.float32)
    xT_all[:, :N_NODES] = np.asarray(x, np.float32).T
    iota_m = np.broadcast_to(np.arange(128, dtype=np.float32), (128, 128)).astype(BF16)
    cb = np.zeros((128, 3 * FO), np.float32)
    cb[:, 0:FO] = c1; cb[:, FO:2 * FO] = c2; cb[:, 2 * FO:3 * FO] = c3
    in_maps = []
    for c in range(N_CORES):
        rows = slice(c * PER_CORE, (c + 1) * PER_CORE)
        din = dinv_pad[rows].reshape(N_WIN, 128).T.copy()  # [128, N_WIN]
        in_map = {
            "xT_own": np.ascontiguousarray(xT_all[:, rows]),
            "dinv_own": din,
            "dinv2_own": din * din,
            "v1_own": v1_pad[rows].reshape(N_WIN, 128).T.copy(),
            "v2_own": v2_pad[rows].reshape(N_WIN, 128).T.copy(),
            "iota": iota_m.copy(),
            "dl_win": dl_win[c].reshape(-1, 128).T.astype(BF16).copy(),
            "W123": W123.copy(),
            "cb": cb.copy(),
        }
        for q in range(NQ):
            in_map[f"idx_q{q}"] = _wrap_idx16(idx_q[q][c])
        in_maps.append(in_map)

    if sim:
        from concourse.bass_interp import MultiCoreSim
        mcs = MultiCoreSim(nc, num_cores=N_CORES, trace=False,
                           require_finite=False, require_nnan=False)
        for ci, core in enumerate(mcs.cores.values()):
            for k, v in in_maps[ci].items():
                core.tensor(k)[:] = v
        mcs.simulate(check_with_hw=False)
        outs = [np.asarray(core.tensor("h_out"))
                for core in mcs.cores.values()]
        res = None
    else:
        res = bass_utils.run_bass_kernel_spmd(
            nc, in_maps, core_ids=list(range(N_CORES)), trace=trace)
        outs = [r["h_out"] for r in res.results]
    full = np.concatenate(outs, axis=0)[:N_NODES]
    return full, res


def kernel(**inputs) -> np.ndarray:
    edge_index = np.asarray(inputs["edge_index"])
    prep = _preprocess(edge_index)
    out, _ = _build_and_run(inputs, *prep)
    return out


# revision 25
# speedup vs baseline: 1.0138x; 1.0021x over previous
"""GCN 3-layer kernel for Trainium2, 8-core SPMD — v8.

Algebraic restructure: with no inter-layer nonlinearity, the network
collapses to

    out = relu( A^3 (X W1W2W3) + (A^2 1) b1'W2W3 + (A 1) b2'W3 + 1 b3' )

where A = D^-1/2 (Adj^T + I) D^-1/2 is the normalized propagation
operator.  W123 = W1@W2@W3 and the rank-1 bias corrections (v1 = A 1,
v2 = A^2 1) are precomputed on the host, so the device only runs THREE
sparse propagations of a 64-wide table plus one dense 128x64 input
matmul — no per-layer GEMMs, no transposes.

Per propagation round (per dst-shard core, PyG GCN convention):
    y      = dinv * T                  (64-wide bf16 message table)
    agg[d] = sum_{e: dst[e]=d} y[src[e]]
    T_next = dinv * agg + dinv^2 * T   (self-loop folded via s-table)

Distribution: destination-sharded across 8 cores (6272 nodes/core).
The y table rows are stored 256B-strided (64 bf16 payload + 64 garbage
pad) because the SWDGE gather stride field has 256B granularity; the
per-edge gather moves only the 128B payload (raw-emitted
InstDMAGatherAnt — the bass wrapper's %256 payload restriction is a
transpose-mode constraint).  Per layer the table is replicated via TWO
AllGathers over window halves (0-24 / 25-48), issued early so they hide
under the previous rounds's gathers; halving also keeps gather indices
below 2^15 (int16).

Each core gathers message rows for its incoming edges with per-edge
SWDGE descriptors on the four Q7 queue-pairs; messages are scatter-added
with one-hot matmuls on the PE (PSUM accumulation per 128-dst window);
the one-hot S for a whole window is built with a single broadcast
tensor_tensor is_equal.
"""

import numpy as np
import ml_dtypes

N_NODES = 50000
N_CORES = 8
PER_CORE = 6272            # 49 * 128
N_PAD = PER_CORE * N_CORES # 50176
N_WIN = 49
QUARTERS = [(0, 25), (25, 49)]  # window ranges per table half
NQ = 2
F = 128                    # input feature width / padded row stride
FO = 64                    # propagated table width (= output width)
GROUP_WINDOWS = 5          # windows per compute group
# AG for half q fires after this group index (group covers its windows)
AG_AFTER_GROUP = [4, 9]

BF16 = ml_dtypes.bfloat16


def _wrap_idx16(idx: np.ndarray) -> np.ndarray:
    """Wrap a flat int16 index stream into the [128, n/16] layout dma_gather
    expects (element i at [i%16, i//16], replicated across the 8 groups of
    16 partitions)."""
    n = len(idx)
    assert n % 128 == 0
    cols = n // 16
    out = np.empty((128, cols), np.int16)
    w = idx.reshape(cols, 16).T  # [16, cols]
    for g in range(8):
        out[g * 16:(g + 1) * 16, :] = w
    return out


def _preprocess(edge_index: np.ndarray):
    """Host-side graph prep: degree norm, dst-sharding, per-(window, half)
    edge streams, block padding shared across cores, and the A^k 1 vectors
    for the rank-1 bias corrections.  Self-loops are NOT materialized as
    edges (handled via the s table)."""
    src = edge_index[0].astype(np.int64)
    dst = edge_index[1].astype(np.int64)
    deg = np.bincount(dst, minlength=N_NODES).astype(np.float64) + 1.0
    dinv64 = 1.0 / np.sqrt(deg)
    dinv = dinv64.astype(np.float32)
    dinv_pad = np.ones(N_PAD, np.float32)
    dinv_pad[:N_NODES] = dinv

    # v1 = A 1, v2 = A v1 (host, fp64)
    def ahat(t):
        agg = np.zeros(N_NODES, np.float64)
        np.add.at(agg, dst, dinv64[src] * t[src])
        return dinv64 * agg + dinv64 * dinv64 * t

    v1 = ahat(np.ones(N_NODES, np.float64))
    v2 = ahat(v1)
    v1_pad = np.zeros(N_PAD, np.float32); v1_pad[:N_NODES] = v1
    v2_pad = np.zeros(N_PAD, np.float32); v2_pad[:N_NODES] = v2

    core_of = dst // PER_CORE
    win_of = (dst % PER_CORE) // 128
    dloc_of = dst % 128

    src_core = src // PER_CORE
    src_off = src % PER_CORE
    src_win = src_off // 128
    q_of = np.searchsorted([q[1] for q in QUARTERS], src_win, side="right")
    rows_q = np.array([(q[1] - q[0]) * 128 for q in QUARTERS])
    w0_q = np.array([q[0] * 128 for q in QUARTERS])
    idx_val = src_core * rows_q[q_of] + (src_off - w0_q[q_of])

    order = np.lexsort((dst, q_of, win_of, core_of))
    core_s, win_s, dloc_s, q_s, iv_s = (
        core_of[order], win_of[order], dloc_of[order], q_of[order],
        idx_val[order])

    # per (core, window, half) counts -> shared block counts
    counts = np.zeros((N_CORES, N_WIN, NQ), np.int64)
    np.add.at(counts, (core_s, win_s, q_s), 1)
    blk = np.maximum(1, -(-counts.max(axis=0) // 128))  # [N_WIN, NQ]

    # per-half stream offsets: stream q holds its blocks window-major
    off = np.zeros((NQ, N_WIN + 1), np.int64)
    for q in range(NQ):
        off[q, 1:] = np.cumsum(blk[:, q] * 128)
    n_q = off[:, -1].astype(int)  # slots per stream

    idx_q = [np.zeros((N_CORES, int(n)), np.int16) for n in n_q]
    dl_q = [np.full((N_CORES, int(n)), 999.0, np.float32) for n in n_q]

    keys = (core_s * N_WIN + win_s) * NQ + q_s
    bounds = np.searchsorted(keys, np.arange(N_CORES * N_WIN * NQ + 1))
    for c in range(N_CORES):
        for w in range(N_WIN):
            for q in range(NQ):
                k = (c * N_WIN + w) * NQ + q
                sl = slice(bounds[k], bounds[k + 1])
                iv = iv_s[sl]; dl = dloc_s[sl]
                o = off[q, w]
                idx_q[q][c, o:o + len(iv)] = iv.astype(np.int16)
                dl_q[q][c, o:o + len(iv)] = dl

    # combined per-window dl (all halves' blocks of window w contiguous),
    # matching the matmul consumption order
    blk_w = blk.sum(axis=1)             # blocks per window
    off_w = np.concatenate([[0], np.cumsum(blk_w)])  # block offsets
    n_blk = int(off_w[-1])
    dl_win = np.full((N_CORES, n_blk * 128), 999.0, np.float32)
    for c in range(N_CORES):
        for w in range(N_WIN):
            o = off_w[w] * 128
            for q in range(NQ):
                nbq = int(blk[w, q]) * 128
                dl_win[c, o:o + nbq] = dl_q[q][c, off[q, w]:off[q, w] + nbq]
                o += nbq

    return (dinv_pad, v1_pad, v2_pad, blk, off, idx_q, dl_win, blk_w, off_w)


def _raw_gather(gp, mybir, out_ap, in_ap, idxs_ap, num_idxs, elem_size,
                elem_step, queue_num):
    """dma_gather without the elem_size_bytes % 256 restriction (a
    transpose-mode constraint); the row stride (elem_step) must still be a
    multiple of 256B due to the 8-bit stride_bytes_256 descriptor field."""
    stride_bytes = elem_step * mybir.dt.size(in_ap.dtype)
    assert stride_bytes % 256 == 0 and stride_bytes // 256 < 256
    _in_ap = gp.lower_ap_dma(in_ap, for_custom_bir_dma=True)
    _idxs_ap = gp.lower_ap(idxs_ap)
    _out_ap = gp.lower_ap(out_ap)
    return gp.add_instruction(
        mybir.InstDMAGatherAnt(
            name=gp.bass.get_next_instruction_name(),
            ins=[*_in_ap, _idxs_ap,
                 gp.lower_val_access(gp.to_reg(num_idxs))],
            outs=[_out_ap],
            transpose=False,
            num_idxs=num_idxs,
            elem_size=elem_size,
            stride_bytes_256=stride_bytes // 256,
            gen_mode=0,
            single_packet=False,
            queue_num=queue_num,
            sbuf_tokens_per_rank=0,
            sbuf_free_dim_per_rank=0,
            sbuf_free_dim_pad_per_rank=0,
            sbuf_byte_offset=0,
        ))


def _build_and_run(inputs_np, dinv_pad, v1_pad, v2_pad, blk, off, idx_q,
                   dl_win, blk_w, off_w, trace=False, sim=False):
    import concourse.bacc as bacc
    import concourse.mybir as mybir
    from concourse.tile import TileContext
    from concourse import bass, bass_utils, library_config

    x = inputs_np["x"]
    W1 = np.asarray(inputs_np["W1"], np.float64)
    W2 = np.asarray(inputs_np["W2"], np.float64)
    W3 = np.asarray(inputs_np["W3"], np.float64)
    b1 = np.asarray(inputs_np["b1"], np.float64)
    b2 = np.asarray(inputs_np["b2"], np.float64)
    b3 = np.asarray(inputs_np["b3"], np.float64)
    W123 = (W1 @ W2 @ W3).astype(np.float32)          # [128, 64]
    c1 = (b1 @ W2 @ W3).astype(np.float32)            # [64]
    c2 = (b2 @ W3).astype(np.float32)                 # [64]
    c3 = b3.astype(np.float32)                        # [64]

    n_q = [int(idx_q[q].shape[1]) for q in range(NQ)]
    n_blk = int(off_w[-1])
    G = GROUP_WINDOWS
    groups = [list(range(g, min(g + G, N_WIN))) for g in range(0, N_WIN, G)]
    rows_q = [(q1 - q0) * 128 for q0, q1 in QUARTERS]
    # per (group, half) slot ranges, each split into two block-balanced subs
    def subsplit(q0, q1):
        mid = q0 + ((q1 - q0) // 256) * 128
        return ((q0, mid), (mid, q1))
    gr = [[subsplit(int(off[q, g[0]]), int(off[q, g[-1] + 1]))
           for q in range(NQ)] for g in groups]
    cap = [[max(r_[q][i][1] - r_[q][i][0] for r_ in gr) // 128
            for i in range(2)] for q in range(NQ)]

    nc = bacc.Bacc("TRN2", target_bir_lowering=False, debug=False,
                   num_devices=N_CORES, num_swdge_queues=4)
    dt = mybir.dt
    Alu = mybir.AluOpType
    Act = mybir.ActivationFunctionType

    # ---- kernel I/O -----------------------------------------------------
    t_xT = nc.dram_tensor("xT_own", [128, PER_CORE], dt.float32, kind="ExternalInput")
    t_W = nc.dram_tensor("W123", [F, FO], dt.float32, kind="ExternalInput")
    t_cb = nc.dram_tensor("cb", [128, 3 * FO], dt.float32, kind="ExternalInput")
    t_dinv = nc.dram_tensor("dinv_own", [128, N_WIN], dt.float32, kind="ExternalInput")
    t_dinv2 = nc.dram_tensor("dinv2_own", [128, N_WIN], dt.float32, kind="ExternalInput")
    t_v1 = nc.dram_tensor("v1_own", [128, N_WIN], dt.float32, kind="ExternalInput")
    t_v2 = nc.dram_tensor("v2_own", [128, N_WIN], dt.float32, kind="ExternalInput")
    t_iota = nc.dram_tensor("iota", [128, 128], dt.bfloat16, kind="ExternalInput")
    t_iq = [nc.dram_tensor(f"idx_q{q}", [128, n_q[q] // 16], dt.int16,
                           kind="ExternalInput") for q in range(NQ)]
    t_dlw = nc.dram_tensor("dl_win", [128, n_blk], dt.bfloat16, kind="ExternalInput")
    t_out = nc.dram_tensor("h_out", [PER_CORE, FO], dt.float32, kind="ExternalOutput")

    with TileContext(nc) as tc:
        nc.gpsimd.load_library(library_config.mlp)
        with tc.tile_pool(name="const", bufs=1) as cpool, \
             tc.tile_pool(name="state", bufs=1) as spool, \
             tc.tile_pool(name="gath", bufs=2) as gpool, \
             tc.tile_pool(name="sbld", bufs=6) as sbld, \
             tc.tile_pool(name="work", bufs=3) as wpool, \
             tc.tile_pool(name="stg", bufs=2) as stg, \
             tc.tile_pool(name="psA", bufs=4, space="PSUM") as psA, \
             tc.tile_pool(name="dram", bufs=1, space="DRAM") as dpool:

            # ---- constants ----
            c_W = cpool.tile([F, FO], dt.float32, tag="W", name="W123")
            c_cb = cpool.tile([128, 3 * FO], dt.float32, tag="cb", name="cb")
            c_dinv = cpool.tile([128, N_WIN], dt.float32, tag="dinv", name="dinv")
            c_dinv2 = cpool.tile([128, N_WIN], dt.float32, tag="dinv2", name="dinv2")
            c_v1 = cpool.tile([128, N_WIN], dt.float32, tag="v1", name="v1")
            c_v2 = cpool.tile([128, N_WIN], dt.float32, tag="v2", name="v2")
            c_iota = cpool.tile([128, 128], dt.bfloat16, tag="iota", name="iota")
            c_iq = [cpool.tile([128, n_q[q] // 16], dt.int16, tag=f"iq{q}",
                               name=f"iq{q}") for q in range(NQ)]
            c_dlw = cpool.tile([128, n_blk], dt.bfloat16, tag="dlw", name="dlw")
            nc.sync.dma_start(c_W[:], t_W[:])
            nc.sync.dma_start(c_cb[:], t_cb[:])
            nc.sync.dma_start(c_dinv[:], t_dinv[:])
            nc.sync.dma_start(c_dinv2[:], t_dinv2[:])
            nc.sync.dma_start(c_v1[:], t_v1[:])
            nc.sync.dma_start(c_v2[:], t_v2[:])
            nc.sync.dma_start(c_iota[:], t_iota[:])
            for q in range(NQ):
                nc.sync.dma_start(c_iq[q][:], t_iq[q][:])
            nc.sync.dma_start(c_dlw[:], t_dlw[:])

            # ---- persistent state: s = dinv^2 * T (+c3 for round 3) ------
            s_tab = [spool.tile([128, N_WIN, FO], dt.float32, tag="s_a", name="s_a"),
                     spool.tile([128, N_WIN, FO], dt.float32, tag="s_b", name="s_b")]

            # y tables: rows 256B-strided, payload = first 64 bf16
            y_full = [[dpool.tile([N_CORES * rows_q[q], F], dt.bfloat16,
                                  addr_space="Shared", name=f"y_full{i}_{q}")
                       for q in range(NQ)] for i in range(3)]
            ag_in = [[dpool.tile([rows_q[q], F], dt.bfloat16, name=f"ag_in{i}_{q}")
                      for q in range(NQ)] for i in range(3)]

            def flush_y(g, r_next, yst):
                """DMA the staged bf16 y rows of group g to the AG inputs.
                A group may straddle a half boundary."""
                w0, w1 = g[0], g[-1] + 1
                s = w0
                while s < w1:
                    q = next(i for i, (a, b) in enumerate(QUARTERS)
                             if a <= s < b)
                    e = min(w1, QUARTERS[q][1])
                    dst = ag_in[r_next][q][
                        (s - QUARTERS[q][0]) * 128:(e - QUARTERS[q][0]) * 128, :]
                    nc.sync.dma_start(dst.rearrange("(t p) f -> p t f", p=128),
                                      yst[:, s - w0:e - w0, :])
                    s = e

            def ag_half(r, q):
                nc.gpsimd.collective_compute(
                    "AllGather", Alu.bypass,
                    replica_groups=[list(range(N_CORES))],
                    ins=[ag_in[r][q].opt()], outs=[y_full[r][q].opt()])

            def gather_sub(dst_tile, r, gi, q, i):
                q0, q1 = gr[gi][q][i]
                nq = q1 - q0
                _raw_gather(nc.gpsimd, mybir,
                            dst_tile[:, :nq // 128, :],
                            y_full[r][q][:, :FO],
                            c_iq[q][:, q0 // 16:q1 // 16],
                            nq, FO, F, 2 * q + i)

            def prefetch_round(prefetch, r):
                """Gather round r's first stream-0 chunks into the tail of
                the previous round/phase (its half-0 AG has already fired)."""
                for pgi in (0, 1):
                    for i in range(2):
                        t = gpool.tile([128, cap[0][i], FO], dt.bfloat16,
                                       tag=f"pf{pgi}_{i}", name=f"pf{pgi}_{i}")
                        gather_sub(t, r, pgi, 0, i)
                        prefetch[(r, pgi, i)] = t

            prefetch = {}  # (r, gi, sub) -> stream-0 tile gathered early

            # ---- phase 0: T1 = A-ready tables from Z = x @ W123 ----------
            with tc.tile_pool(name="xp", bufs=1) as xpool:
                xT = xpool.tile([128, PER_CORE], dt.float32, tag="xT", name="xT")
                nc.sync.dma_start(xT[:], t_xT[:])
                nags = 0
                for gi, g in enumerate(groups):
                    yst = stg.tile([128, G, F], dt.bfloat16, tag="yst", name="yst")
                    for wi, w in enumerate(g):
                        ps = psA.tile([128, FO], dt.float32, tag="psA", space="PSUM")
                        nc.tensor.matmul(ps[:], lhsT=xT[:, w * 128:(w + 1) * 128],
                                         rhs=c_W[:], start=True, stop=True)
                        # y1 = dinv * Z (bf16 payload), s1 = dinv^2 * Z
                        nc.scalar.mul(yst[:, wi, :FO], ps[:], c_dinv[:, w:w + 1])
                        nc.scalar.mul(s_tab[1][:, w, :], ps[:], c_dinv2[:, w:w + 1])
                    flush_y(g, 0, yst)
                    while nags < NQ and AG_AFTER_GROUP[nags] == gi:
                        ag_half(0, nags)
                        nags += 1
                    if gi == 9:
                        prefetch_round(prefetch, 0)

            # ---- propagation rounds --------------------------------------
            for r in range(3):
                nags = 0
                for gi, g in enumerate(groups):
                    m_q = [[None, None] for q in range(NQ)]
                    for q in range(NQ):
                        for i in range(2):
                            if q == 0 and (r, gi, i) in prefetch:
                                m_q[q][i] = prefetch.pop((r, gi, i))
                                continue
                            m_q[q][i] = gpool.tile(
                                [128, cap[q][i], FO], dt.bfloat16,
                                tag=f"m{q}_{i}", name=f"m{q}_{i}")
                            gather_sub(m_q[q][i], r, gi, q, i)
                    # during the last group, prefetch next round's first
                    # stream-0 gathers into the round-boundary bubble
                    if r < 2 and gi == 9:
                        prefetch_round(prefetch, r + 1)
                    yst = stg.tile([128, G, F], dt.bfloat16, tag="yst", name="yst")
                    ost = stg.tile([128, G, FO], dt.float32, tag="ost", name="ost")
                    for wi, w in enumerate(g):
                        nblk = int(blk_w[w])
                        B0 = int(off_w[w])
                        # one-hot S for the whole window in one op
                        S = sbld.tile([128, nblk, 128], dt.bfloat16, tag="S", name="S")
                        dl_b = (c_dlw[:, B0:B0 + nblk].unsqueeze(2)
                                .broadcast_to([128, nblk, 128]))
                        nc.vector.tensor_tensor(
                            out=S[:, :, :], in0=dl_b,
                            in1=c_iota[:].unsqueeze(1).broadcast_to([128, nblk, 128]),
                            op=Alu.is_equal)
                        # scatter-add via PSUM-accumulated one-hot matmuls
                        agg = psA.tile([128, FO], dt.float32, tag="psA", space="PSUM")
                        k = 0
                        for q in range(NQ):
                            for b in range(int(blk[w, q])):
                                gslot = int(off[q, w]) + b * 128
                                sub = 0 if gslot < gr[gi][q][0][1] else 1
                                Bq = (gslot - gr[gi][q][sub][0]) // 128
                                nc.tensor.matmul(
                                    agg[:], lhsT=S[:, k, :],
                                    rhs=m_q[q][sub][:, Bq, :],
                                    start=(k == 0), stop=(k == nblk - 1))
                                k += 1
                        # ---- epilogue: T = dinv*agg + s ----
                        h = wpool.tile([128, FO], dt.float32, tag="h", name="h")
                        nc.vector.scalar_tensor_tensor(
                            out=h[:], in0=agg[:], scalar=c_dinv[:, w:w + 1],
                            in1=s_tab[(r + 1) % 2][:, w, :],
                            op0=Alu.mult, op1=Alu.add)
                        if r < 2:
                            nc.scalar.mul(yst[:, wi, :FO], h[:], c_dinv[:, w:w + 1])
                            if r == 0:
                                nc.scalar.mul(s_tab[0][:, w, :], h[:],
                                              c_dinv2[:, w:w + 1])
                            else:
                                # s3 = dinv^2*T3 + c3 (bias const folded in)
                                nc.vector.scalar_tensor_tensor(
                                    out=s_tab[1][:, w, :], in0=h[:],
                                    scalar=c_dinv2[:, w:w + 1],
                                    in1=c_cb[:, 2 * FO:3 * FO],
                                    op0=Alu.mult, op1=Alu.add)
                        else:
                            # out = relu(T4 + v1*c2 + v2*c1)
                            f1 = wpool.tile([128, FO], dt.float32, tag="f1", name="f1")
                            nc.vector.scalar_tensor_tensor(
                                out=f1[:], in0=c_cb[:, FO:2 * FO],
                                scalar=c_v1[:, w:w + 1], in1=h[:],
                                op0=Alu.mult, op1=Alu.add)
                            f2 = wpool.tile([128, FO], dt.float32, tag="f2", name="f2")
                            nc.vector.scalar_tensor_tensor(
                                out=f2[:], in0=c_cb[:, 0:FO],
                                scalar=c_v2[:, w:w + 1], in1=f1[:],
                                op0=Alu.mult, op1=Alu.add)
                            nc.scalar.activation(ost[:, wi, :], f2[:], Act.Relu)
                    if r < 2:
                        flush_y(g, r + 1, yst)
                        while nags < NQ and AG_AFTER_GROUP[nags] == gi:
                            ag_half(r + 1, nags)
                            nags += 1
                    else:
                        w0, w1 = g[0], g[-1] + 1
                        nc.sync.dma_start(
                            t_out[w0 * 128:w1 * 128, :]
                            .rearrange("(t p) f -> p t f", p=128),
                            ost[:, :w1 - w0, :])

    nc.compile()

    # ---- per-core inputs ----
    xT_all = np.zeros((128, N_PAD), np.float32)
    xT_all[:, :N_NODES] = np.asarray(x, np.float32).T
    iota_m = np.broadcast_to(np.arange(128, dtype=np.float32), (128, 128)).astype(BF16)
    cb = np.zeros((128, 3 * FO), np.float32)
    cb[:, 0:FO] = c1; cb[:, FO:2 * FO] = c2; cb[:, 2 * FO:3 * FO] = c3
    in_maps = []
    for c in range(N_CORES):
        rows = slice(c * PER_CORE, (c + 1) * PER_CORE)
        din = dinv_pad[rows].reshape(N_WIN, 128).T.copy()  # [128, N_WIN]
        in_map = {
            "xT_own": np.ascontiguousarray(xT_all[:, rows]),
            "dinv_own": din,
            "dinv2_own": din * din,
            "v1_own": v1_pad[rows].reshape(N_WIN, 128).T.copy(),
            "v2_own": v2_pad[rows].reshape(N_WIN, 128).T.copy(),
            "iota": iota_m.copy(),
            "dl_win": dl_win[c].reshape(-1, 128).T.astype(BF16).copy(),
            "W123": W123.copy(),
            "cb": cb.copy(),
        }
        for q in range(NQ):
            in_map[f"idx_q{q}"] = _wrap_idx16(idx_q[q][c])
        in_maps.append(in_map)

    if sim:
        from concourse.bass_interp import MultiCoreSim
        mcs = MultiCoreSim(nc, num_cores=N_CORES, trace=False,
                           require_finite=False, require_nnan=False)
        for ci, core in enumerate(mcs.cores.values()):
            for k, v in in_maps[ci].items():
                core.tensor(k)[:] = v
        mcs.simulate(check_with_hw=False)
        outs = [np.asarray(core.tensor("h_out"))
                for core in mcs.cores.values()]
        res = None
    else:
        res = bass_utils.run_bass_kernel_spmd(
            nc, in_maps, core_ids=list(range(N_CORES)), trace=trace)
        outs = [r["h_out"] for r in res.results]
    full = np.concatenate(outs, axis=0)[:N_NODES]
    return full, res


def kernel(**inputs) -> np.ndarray:
    edge_index = np.asarray(inputs["edge_index"])
    prep = _preprocess(edge_index)
    out, _ = _build_and_run(inputs, *prep)
    return out


# revision 37
# speedup vs baseline: 1.1508x; 1.1352x over previous
"""GCN 3-layer kernel for Trainium2, 8-core SPMD — v8.

Algebraic restructure: with no inter-layer nonlinearity, the network
collapses to

    out = relu( A^3 (X W1W2W3) + (A^2 1) b1'W2W3 + (A 1) b2'W3 + 1 b3' )

where A = D^-1/2 (Adj^T + I) D^-1/2 is the normalized propagation
operator.  W123 = W1@W2@W3 and the rank-1 bias corrections (v1 = A 1,
v2 = A^2 1) are precomputed on the host, so the device only runs THREE
sparse propagations of a 64-wide table plus one dense 128x64 input
matmul — no per-layer GEMMs, no transposes.

Per propagation round (per dst-shard core, PyG GCN convention):
    y      = dinv * T                  (64-wide bf16 message table)
    agg[d] = sum_{e: dst[e]=d} y[src[e]]
    T_next = dinv * agg + dinv^2 * T   (self-loop folded via s-table)

Distribution: destination-sharded across 8 cores (6272 nodes/core).
The y table rows are stored 256B-strided (64 bf16 payload + 64 garbage
pad) because the SWDGE gather stride field has 256B granularity; the
per-edge gather moves only the 128B payload (raw-emitted
InstDMAGatherAnt — the bass wrapper's %256 payload restriction is a
transpose-mode constraint).  Per layer the table is replicated via TWO
AllGathers over window halves (0-24 / 25-48), issued early so they hide
under the previous round's gathers; halving also keeps gather indices
below 2^15 (int16).

Each core gathers message rows for its incoming edges with per-edge
SWDGE descriptors on the four Q7 queue-pairs; messages are scatter-added
with one-hot matmuls on the PE (PSUM accumulation per 128-dst window);
the one-hot S for a whole window is built with a single broadcast
tensor_tensor is_equal.
"""

import numpy as np
import ml_dtypes

N_NODES = 50000
N_CORES = 8
PER_CORE = 6272            # 49 * 128
N_PAD = PER_CORE * N_CORES # 50176
N_WIN = 49
QUARTERS = [(0, 25), (25, 49)]  # window ranges per table half
NQ = 2
F = 128                    # input feature width / padded row stride
FO = 64                    # propagated table width (= output width)
GROUP_WINDOWS = 5          # windows per gather chunk
# AG for half q fires after this group index (group covers its windows)
AG_AFTER_GROUP = [4, 9]

BF16 = ml_dtypes.bfloat16


def _wrap_idx16(idx: np.ndarray) -> np.ndarray:
    """Wrap a flat int16 index stream into the [128, n/16] layout dma_gather
    expects (element i at [i%16, i//16], replicated across the 8 groups of
    16 partitions)."""
    n = len(idx)
    assert n % 128 == 0
    cols = n // 16
    out = np.empty((128, cols), np.int16)
    w = idx.reshape(cols, 16).T  # [16, cols]
    for g in range(8):
        out[g * 16:(g + 1) * 16, :] = w
    return out


def _preprocess(edge_index: np.ndarray):
    """Host-side graph prep: degree norm, dst-sharding, per-(window, half)
    edge streams, block padding shared across cores, and the A^k 1 vectors
    for the rank-1 bias corrections.  Self-loops are NOT materialized as
    edges (handled via the s table)."""
    src = edge_index[0].astype(np.int64)
    dst = edge_index[1].astype(np.int64)
    deg = np.bincount(dst, minlength=N_NODES).astype(np.float64) + 1.0
    dinv64 = 1.0 / np.sqrt(deg)
    dinv = dinv64.astype(np.float32)
    dinv_pad = np.ones(N_PAD, np.float32)
    dinv_pad[:N_NODES] = dinv

    # v1 = A 1, v2 = A v1 (host, fp64)
    def ahat(t):
        agg = np.zeros(N_NODES, np.float64)
        np.add.at(agg, dst, dinv64[src] * t[src])
        return dinv64 * agg + dinv64 * dinv64 * t

    v1 = ahat(np.ones(N_NODES, np.float64))
    v2 = ahat(v1)
    v1_pad = np.zeros(N_PAD, np.float32); v1_pad[:N_NODES] = v1
    v2_pad = np.zeros(N_PAD, np.float32); v2_pad[:N_NODES] = v2

    core_of = dst // PER_CORE
    win_of = (dst % PER_CORE) // 128
    dloc_of = dst % 128

    src_core = src // PER_CORE
    src_off = src % PER_CORE
    src_win = src_off // 128
    q_of = np.searchsorted([q[1] for q in QUARTERS], src_win, side="right")
    rows_q = np.array([(q[1] - q[0]) * 128 for q in QUARTERS])
    w0_q = np.array([q[0] * 128 for q in QUARTERS])
    idx_val = src_core * rows_q[q_of] + (src_off - w0_q[q_of])

    order = np.lexsort((dst, q_of, win_of, core_of))
    core_s, win_s, dloc_s, q_s, iv_s = (
        core_of[order], win_of[order], dloc_of[order], q_of[order],
        idx_val[order])

    # per (core, window, half) counts -> shared block counts
    counts = np.zeros((N_CORES, N_WIN, NQ), np.int64)
    np.add.at(counts, (core_s, win_s, q_s), 1)
    blk = np.maximum(1, -(-counts.max(axis=0) // 128))  # [N_WIN, NQ]

    # per-half stream offsets: stream q holds its blocks window-major
    off = np.zeros((NQ, N_WIN + 1), np.int64)
    for q in range(NQ):
        off[q, 1:] = np.cumsum(blk[:, q] * 128)
    n_q = off[:, -1].astype(int)  # slots per stream

    idx_q = [np.zeros((N_CORES, int(n)), np.int16) for n in n_q]
    dl_q = [np.full((N_CORES, int(n)), 999.0, np.float32) for n in n_q]

    keys = (core_s * N_WIN + win_s) * NQ + q_s
    bounds = np.searchsorted(keys, np.arange(N_CORES * N_WIN * NQ + 1))
    for c in range(N_CORES):
        for w in range(N_WIN):
            for q in range(NQ):
                k = (c * N_WIN + w) * NQ + q
                sl = slice(bounds[k], bounds[k + 1])
                iv = iv_s[sl]; dl = dloc_s[sl]
                o = off[q, w]
                idx_q[q][c, o:o + len(iv)] = iv.astype(np.int16)
                dl_q[q][c, o:o + len(iv)] = dl

    # combined per-window dl (all halves' blocks of window w contiguous),
    # matching the matmul consumption order
    blk_w = blk.sum(axis=1)             # blocks per window
    off_w = np.concatenate([[0], np.cumsum(blk_w)])  # block offsets
    n_blk = int(off_w[-1])
    dl_win = np.full((N_CORES, n_blk * 128), 999.0, np.float32)
    for c in range(N_CORES):
        for w in range(N_WIN):
            o = off_w[w] * 128
            for q in range(NQ):
                nbq = int(blk[w, q]) * 128
                dl_win[c, o:o + nbq] = dl_q[q][c, off[q, w]:off[q, w] + nbq]
                o += nbq

    return (dinv_pad, v1_pad, v2_pad, blk, off, idx_q, dl_win, blk_w, off_w)


def _raw_gather(gp, mybir, out_ap, in_ap, idxs_ap, num_idxs, elem_size,
                elem_step, queue_num):
    """dma_gather without the elem_size_bytes % 256 restriction (a
    transpose-mode constraint); the row stride (elem_step) must still be a
    multiple of 256B due to the 8-bit stride_bytes_256 descriptor field."""
    stride_bytes = elem_step * mybir.dt.size(in_ap.dtype)
    assert stride_bytes % 256 == 0 and stride_bytes // 256 < 256
    _in_ap = gp.lower_ap_dma(in_ap, for_custom_bir_dma=True)
    _idxs_ap = gp.lower_ap(idxs_ap)
    _out_ap = gp.lower_ap(out_ap)
    return gp.add_instruction(
        mybir.InstDMAGatherAnt(
            name=gp.bass.get_next_instruction_name(),
            ins=[*_in_ap, _idxs_ap,
                 gp.lower_val_access(gp.to_reg(num_idxs))],
            outs=[_out_ap],
            transpose=False,
            num_idxs=num_idxs,
            elem_size=elem_size,
            stride_bytes_256=stride_bytes // 256,
            gen_mode=0,
            single_packet=False,
            queue_num=queue_num,
            sbuf_tokens_per_rank=0,
            sbuf_free_dim_per_rank=0,
            sbuf_free_dim_pad_per_rank=0,
            sbuf_byte_offset=0,
        ))


def _build_and_run(inputs_np, dinv_pad, v1_pad, v2_pad, blk, off, idx_q,
                   dl_win, blk_w, off_w, trace=False, sim=False):
    import concourse.bacc as bacc
    import concourse.mybir as mybir
    from concourse.tile import TileContext
    from concourse import bass, bass_utils, library_config

    x = inputs_np["x"]
    W1 = np.asarray(inputs_np["W1"], np.float64)
    W2 = np.asarray(inputs_np["W2"], np.float64)
    W3 = np.asarray(inputs_np["W3"], np.float64)
    b1 = np.asarray(inputs_np["b1"], np.float64)
    b2 = np.asarray(inputs_np["b2"], np.float64)
    b3 = np.asarray(inputs_np["b3"], np.float64)
    W123 = (W1 @ W2 @ W3).astype(np.float32)          # [128, 64]
    c1 = (b1 @ W2 @ W3).astype(np.float32)            # [64]
    c2 = (b2 @ W3).astype(np.float32)                 # [64]
    c3 = b3.astype(np.float32)                        # [64]

    n_q = [int(idx_q[q].shape[1]) for q in range(NQ)]
    n_blk = int(off_w[-1])
    G = GROUP_WINDOWS
    groups = [list(range(g, min(g + G, N_WIN))) for g in range(0, N_WIN, G)]
    # per (group, half) slot ranges, each split into two block-balanced subs
    def subsplit(q0, q1):
        mid = q0 + ((q1 - q0) // 256) * 128
        return ((q0, mid), (mid, q1))
    gr = [[subsplit(int(off[q, g[0]]), int(off[q, g[-1] + 1]))
           for q in range(NQ)] for g in groups]
    cap = [[max(r[q][i][1] - r[q][i][0] for r in gr) // 128 for i in range(2)]
           for q in range(NQ)]
    rows_q = [(q1 - q0) * 128 for q0, q1 in QUARTERS]

    nc = bacc.Bacc("TRN2", target_bir_lowering=False, debug=False,
                   num_devices=N_CORES, num_swdge_queues=4)
    dt = mybir.dt
    Alu = mybir.AluOpType
    Act = mybir.ActivationFunctionType

    # ---- kernel I/O -----------------------------------------------------
    t_xT = nc.dram_tensor("xT_own", [128, PER_CORE], dt.float32, kind="ExternalInput")
    t_W = nc.dram_tensor("W123", [F, FO], dt.float32, kind="ExternalInput")
    t_cb = nc.dram_tensor("cb", [128, 3 * FO], dt.float32, kind="ExternalInput")
    t_dinv = nc.dram_tensor("dinv_own", [128, N_WIN], dt.float32, kind="ExternalInput")
    t_dinv2 = nc.dram_tensor("dinv2_own", [128, N_WIN], dt.float32, kind="ExternalInput")
    t_v1 = nc.dram_tensor("v1_own", [128, N_WIN], dt.float32, kind="ExternalInput")
    t_v2 = nc.dram_tensor("v2_own", [128, N_WIN], dt.float32, kind="ExternalInput")
    t_iota = nc.dram_tensor("iota", [128, 128], dt.bfloat16, kind="ExternalInput")
    t_iq = [nc.dram_tensor(f"idx_q{q}", [128, n_q[q] // 16], dt.int16,
                           kind="ExternalInput") for q in range(NQ)]
    t_dlw = nc.dram_tensor("dl_win", [128, n_blk], dt.bfloat16, kind="ExternalInput")
    t_out = nc.dram_tensor("h_out", [PER_CORE, FO], dt.float32, kind="ExternalOutput")

    with TileContext(nc) as tc:
        nc.gpsimd.load_library(library_config.mlp)
        with tc.tile_pool(name="const", bufs=1) as cpool, \
             tc.tile_pool(name="state", bufs=1) as spool, \
             tc.tile_pool(name="gath", bufs=2) as gpool, \
             tc.tile_pool(name="sbld", bufs=6) as sbld, \
             tc.tile_pool(name="work", bufs=3) as wpool, \
             tc.tile_pool(name="stg", bufs=2) as stg, \
             tc.tile_pool(name="psA", bufs=4, space="PSUM") as psA, \
             tc.tile_pool(name="dram", bufs=1, space="DRAM") as dpool:

            # ---- constants ----
            c_W = cpool.tile([F, FO], dt.float32, tag="W", name="W123")
            c_cb = cpool.tile([128, 3 * FO], dt.float32, tag="cb", name="cb")
            c_dinv = cpool.tile([128, N_WIN], dt.float32, tag="dinv", name="dinv")
            c_dinv2 = cpool.tile([128, N_WIN], dt.float32, tag="dinv2", name="dinv2")
            c_v1 = cpool.tile([128, N_WIN], dt.float32, tag="v1", name="v1")
            c_v2 = cpool.tile([128, N_WIN], dt.float32, tag="v2", name="v2")
            c_iota = cpool.tile([128, 128], dt.bfloat16, tag="iota", name="iota")
            c_iq = [cpool.tile([128, n_q[q] // 16], dt.int16, tag=f"iq{q}",
                               name=f"iq{q}") for q in range(NQ)]
            c_dlw = cpool.tile([128, n_blk], dt.bfloat16, tag="dlw", name="dlw")
            nc.sync.dma_start(c_W[:], t_W[:])
            nc.sync.dma_start(c_cb[:], t_cb[:])
            nc.sync.dma_start(c_dinv[:], t_dinv[:])
            nc.sync.dma_start(c_dinv2[:], t_dinv2[:])
            nc.sync.dma_start(c_v1[:], t_v1[:])
            nc.sync.dma_start(c_v2[:], t_v2[:])
            nc.sync.dma_start(c_iota[:], t_iota[:])
            for q in range(NQ):
                nc.sync.dma_start(c_iq[q][:], t_iq[q][:])
            nc.sync.dma_start(c_dlw[:], t_dlw[:])

            # ---- persistent state: s = dinv^2 * T (+c3 for round 3) ------
            s_tab = [spool.tile([128, N_WIN, FO], dt.float32, tag="s_a", name="s_a"),
                     spool.tile([128, N_WIN, FO], dt.float32, tag="s_b", name="s_b")]

            # y tables: rows 256B-strided, payload = first 64 bf16
            y_full = [[dpool.tile([N_CORES * rows_q[q], F], dt.bfloat16,
                                  addr_space="Shared", name=f"y_full{i}_{q}")
                       for q in range(NQ)] for i in range(3)]
            ag_in = [[dpool.tile([rows_q[q], F], dt.bfloat16, name=f"ag_in{i}_{q}")
                      for q in range(NQ)] for i in range(3)]

            def flush_y(g, r_next, yst):
                """DMA the staged bf16 y rows of group g to the AG inputs.
                A group may straddle a half boundary."""
                w0, w1 = g[0], g[-1] + 1
                s = w0
                while s < w1:
                    q = next(i for i, (a, b) in enumerate(QUARTERS)
                             if a <= s < b)
                    e = min(w1, QUARTERS[q][1])
                    dst = ag_in[r_next][q][
                        (s - QUARTERS[q][0]) * 128:(e - QUARTERS[q][0]) * 128, :]
                    nc.sync.dma_start(dst.rearrange("(t p) f -> p t f", p=128),
                                      yst[:, s - w0:e - w0, :])
                    s = e

            def ag_half(r, q):
                nc.gpsimd.collective_compute(
                    "AllGather", Alu.bypass,
                    replica_groups=[list(range(N_CORES))],
                    ins=[ag_in[r][q].opt()], outs=[y_full[r][q].opt()])

            # ---- phase 0: T1 = A-ready tables from Z = x @ W123 ----------
            with tc.tile_pool(name="xp", bufs=1) as xpool:
                xT = xpool.tile([128, PER_CORE], dt.float32, tag="xT", name="xT")
                nc.sync.dma_start(xT[:], t_xT[:])
                nags = 0
                for gi, g in enumerate(groups):
                    yst = stg.tile([128, G, F], dt.bfloat16, tag="yst", name="yst")
                    for wi, w in enumerate(g):
                        ps = psA.tile([128, FO], dt.float32, tag="psA", space="PSUM")
                        nc.tensor.matmul(ps[:], lhsT=xT[:, w * 128:(w + 1) * 128],
                                         rhs=c_W[:], start=True, stop=True)
                        # y1 = dinv * Z (bf16 payload), s1 = dinv^2 * Z
                        nc.scalar.mul(yst[:, wi, :FO], ps[:], c_dinv[:, w:w + 1])
                        nc.scalar.mul(s_tab[1][:, w, :], ps[:], c_dinv2[:, w:w + 1])
                    flush_y(g, 0, yst)
                    while nags < NQ and AG_AFTER_GROUP[nags] == gi:
                        ag_half(0, nags)
                        nags += 1

            # ---- propagation rounds --------------------------------------
            for r in range(3):
                nags = 0
                for gi, g in enumerate(groups):
                    m_q = [[gpool.tile([128, cap[q][i], FO], dt.bfloat16,
                                       tag=f"m{q}_{i}", name=f"m{q}_{i}")
                            for i in range(2)] for q in range(NQ)]
                    for q in range(NQ):
                        for i in range(2):
                            q0, q1 = gr[gi][q][i]
                            nq = q1 - q0
                            _raw_gather(
                                nc.gpsimd, mybir,
                                m_q[q][i][:, :nq // 128, :],
                                y_full[r][q][:, :FO],
                                c_iq[q][:, q0 // 16:q1 // 16],
                                nq, FO, F, 2 * q + i)
                    yst = stg.tile([128, G, F], dt.bfloat16, tag="yst", name="yst")
                    ost = stg.tile([128, G, FO], dt.float32, tag="ost", name="ost")
                    for wi, w in enumerate(g):
                        nblk = int(blk_w[w])
                        B0 = int(off_w[w])
                        # one-hot S for the whole window in one op
                        S = sbld.tile([128, nblk, 128], dt.bfloat16, tag="S", name="S")
                        dl_b = (c_dlw[:, B0:B0 + nblk].unsqueeze(2)
                                .broadcast_to([128, nblk, 128]))
                        nc.vector.tensor_tensor(
                            out=S[:, :, :], in0=dl_b,
                            in1=c_iota[:].unsqueeze(1).broadcast_to([128, nblk, 128]),
                            op=Alu.is_equal)
                        # scatter-add via PSUM-accumulated one-hot matmuls
                        agg = psA.tile([128, FO], dt.float32, tag="psA", space="PSUM")
                        k = 0
                        for q in range(NQ):
                            for b in range(int(blk[w, q])):
                                gslot = int(off[q, w]) + b * 128
                                sub = 0 if gslot < gr[gi][q][0][1] else 1
                                Bq = (gslot - gr[gi][q][sub][0]) // 128
                                nc.tensor.matmul(
                                    agg[:], lhsT=S[:, k, :],
                                    rhs=m_q[q][sub][:, Bq, :],
                                    start=(k == 0), stop=(k == nblk - 1))
                                k += 1
                        # ---- epilogue: T = dinv*agg + s ----
                        h = wpool.tile([128, FO], dt.float32, tag="h", name="h")
                        nc.vector.scalar_tensor_tensor(
                            out=h[:], in0=agg[:], scalar=c_dinv[:, w:w + 1],
                            in1=s_tab[(r + 1) % 2][:, w, :],
                            op0=Alu.mult, op1=Alu.add)
                        if r < 2:
                            nc.scalar.mul(yst[:, wi, :FO], h[:], c_dinv[:, w:w + 1])
                            if r == 0:
                                nc.scalar.mul(s_tab[0][:, w, :], h[:],
                                              c_dinv2[:, w:w + 1])
                            else:
                                # s3 = dinv^2*T3 + c3 (bias const folded in)
                                nc.vector.scalar_tensor_tensor(
                                    out=s_tab[1][:, w, :], in0=h[:],
                                    scalar=c_dinv2[:, w:w + 1],
                                    in1=c_cb[:, 2 * FO:3 * FO],
                                    op0=Alu.mult, op1=Alu.add)
                        else:
                            # out = relu(T4 + v1*c2 + v2*c1)
                            f1 = wpool.tile([128, FO], dt.float32, tag="f1", name="f1")
                            nc.vector.scalar_tensor_tensor(
                                out=f1[:], in0=c_cb[:, FO:2 * FO],
                                scalar=c_v1[:, w:w + 1], in1=h[:],
                                op0=Alu.mult, op1=Alu.add)
                            f2 = wpool.tile([128, FO], dt.float32, tag="f2", name="f2")
                            nc.vector.scalar_tensor_tensor(
                                out=f2[:], in0=c_cb[:, 0:FO],
                                scalar=c_v2[:, w:w + 1], in1=f1[:],
                                op0=Alu.mult, op1=Alu.add)
                            nc.scalar.activation(ost[:, wi, :], f2[:], Act.Relu)
                    if r < 2:
                        flush_y(g, r + 1, yst)
                        while nags < NQ and AG_AFTER_GROUP[nags] == gi:
                            ag_half(r + 1, nags)
                            nags += 1
                    else:
                        w0, w1 = g[0], g[-1] + 1
                        nc.sync.dma_start(
                            t_out[w0 * 128:w1 * 128, :]
                            .rearrange("(t p) f -> p t f", p=128),
                            ost[:, :w1 - w0, :])

    nc.compile()

    # ---- per-core inputs ----
    xT_all = np.zeros((128, N_PAD), np.float32)
    xT_all[:, :N_NODES] = np.asarray(x, np.float32).T
    iota_m = np.broadcast_to(np.arange(128, dtype=np.float32), (128, 128)).astype(BF16)
    cb = np.zeros((128, 3 * FO), np.float32)
    cb[:, 0:FO] = c1; cb[:, FO:2 * FO] = c2; cb[:, 2 * FO:3 * FO] = c3
    in_maps = []
    for c in range(N_CORES):
        rows = slice(c * PER_CORE, (c + 1) * PER_CORE)
        din = dinv_pad[rows].reshape(N_WIN, 128).T.copy()  # [128, N_WIN]
        in_map = {
            "xT_own": np.ascontiguousarray(xT_all[:, rows]),
            "dinv_own": din,
            "dinv2_own": din * din,
            "v1_own": v1_pad[rows].reshape(N_WIN, 128).T.copy(),
            "v2_own": v2_pad[rows].reshape(N_WIN, 128).T.copy(),
            "iota": iota_m.copy(),
            "dl_win": dl_win[c].reshape(-1, 128).T.astype(BF16).copy(),
            "W123": W123.copy(),
            "cb": cb.copy(),
        }
        for q in range(NQ):
            in_map[f"idx_q{q}"] = _wrap_idx16(idx_q[q][c])
        in_maps.append(in_map)

    if sim:
        from concourse.bass_interp import MultiCoreSim
        mcs = MultiCoreSim(nc, num_cores=N_CORES, trace=False,
                           require_finite=False, require_nnan=False)
        for ci, core in enumerate(mcs.cores.values()):
            for k, v in in_maps[ci].items():
                core.tensor(k)[:] = v
        mcs.simulate(check_with_hw=False)
        outs = [np.asarray(core.tensor("h_out"))
                for core in mcs.cores.values()]
        res = None
    else:
        res = bass_utils.run_bass_kernel_spmd(
            nc, in_maps, core_ids=list(range(N_CORES)), trace=trace)
        outs = [r["h_out"] for r in res.results]
    full = np.concatenate(outs, axis=0)[:N_NODES]
    return full, res


def kernel(**inputs) -> np.ndarray:
    edge_index = np.asarray(inputs["edge_index"])
    prep = _preprocess(edge_index)
    out, _ = _build_and_run(inputs, *prep)
    return out
